# revision 28
# baseline (speedup 1.0000x reference)
"""Trainium2 Bass kernel for a 16-head MHA layer (batch 4, seq 2048, embed 1024).

Sharding: 8 cores; core c handles batch c//2 and query-token half c%2, with the
core's x rotated so its 1024 query tokens sit in rows 0:1024 (softmax is
permutation-invariant over key order). K/V cover the full sequence on-core; no
collectives. Weights replicated.

Numerics: every hot matmul runs in fp8e4 DoubleRow (2 contraction slots per
matmul at 0.5 cycles/output-column), with the residual-pair trick soaking up
fp8 quantization error wherever a slot is free:
 - QK projections contract (W_hi, W_lo) fp8 pairs of 32*Wqkv (scaled out of
   the fp8-subnormal range; host-prepared).
 - Scores contract d=64 with slots (Q_hi, Q_lo) -- the Q store residual --
   against a stride-0-duplicated K.
 - AV contracts slots ([ones/32|V_hi], [ones/32|V_lo]) against stride-0-
   duplicated P, so each matmul emits denominator/16 in rows 0:64 and the
   V-residual-corrected AV in rows 64:128.
 - V projection and the output projection run in bf16 (x^T is uploaded twice:
   fp8 for QK chains, bf16 for V chains; out-proj reads bf16 AT tiles).
 - K bias is dropped (softmax-invariant); V bias + output bias fold into a
   host-precomputed boB = b_v @ Wo + bo; Q bias (x32) is added on-chip.
 - exp: mostly exact on the ACT engine (fp8 out, scale folded); a tunable set
   of key-tiles runs a one-instruction DVE Schraudolph (int8 bit pattern) exp
   to split the exp load across both engines.
"""

import sys

for _p in ("/opt/trn_rl_repo",):
    if _p not in sys.path:
        sys.path.insert(0, _p)

import numpy as np

import concourse.bass as bass  # noqa: E402
import concourse.mybir as mybir  # noqa: E402
import concourse.tile as tile  # noqa: E402
from concourse import bacc  # noqa: E402

SEQ = 2048
E = 1024
H = 16
D = 64
NQ = 1024  # query tokens per core
N_CORES = 8

ET = E // 128  # 8 e-chunks
TT = SEQ // 128  # 16 key/token tiles
HP = H // 2  # 8 head pairs
QB = NQ // 512  # 2 query blocks

F32 = mybir.dt.float32
BF16 = mybir.dt.bfloat16
FP8 = mybir.dt.float8e4
I8 = mybir.dt.int8
AF = mybir.ActivationFunctionType
PM = mybir.MatmulPerfMode

# exp(s_fp8 * SC) == exp(s_true * 0.125); fp8 QK weights carry 32x each
SC = 0.125 / 1024.0
# Schraudolph: int8 bits = round(s_fp8*C1 + C2) reinterpreted as fp8e4
C1 = 8 * 1.4426950408889634 * SC
C2 = 56.0 - 8 * 0.043


def _dve_kts(b):
    if b in (0, 8):  # these blocks' DVE is busy with inner V copies
        return ()
    return (2, 5, 8, 11, 14)


def build_program():
    nc = bacc.Bacc(trn_type="TRN2", target_bir_lowering=False, debug=False)

    xT8 = nc.dram_tensor("xT8", [E, SEQ], FP8, kind="ExternalInput").ap()
    xTb = nc.dram_tensor("xTb", [E, SEQ], BF16, kind="ExternalInput").ap()
    wqk_hi = nc.dram_tensor("wqk_hi", [E, 2 * E], FP8, kind="ExternalInput").ap()
    wqk_lo = nc.dram_tensor("wqk_lo", [E, 2 * E], FP8, kind="ExternalInput").ap()
    wv_b = nc.dram_tensor("wv_b", [E, E], BF16, kind="ExternalInput").ap()
    wo_b = nc.dram_tensor("wo_b", [E, E], BF16, kind="ExternalInput").ap()
    bqT = nc.dram_tensor("bqT", [128, ET], F32, kind="ExternalInput").ap()
    boB = nc.dram_tensor("boB", [E], F32, kind="ExternalInput").ap()
    out = nc.dram_tensor("out", [NQ, E], F32, kind="ExternalOutput").ap()

    with tile.TileContext(nc) as tc:
        _body(nc, tc, xT8, xTb, wqk_hi, wqk_lo, wv_b, wo_b, bqT, boB, out)

    nc.compile()
    return nc


def _body(nc, tc, xT8_d, xTb_d, wqkh_d, wqkl_d, wvb_d, wob_d, bqT_d, boB_d, out):
    from contextlib import ExitStack

    es = ExitStack()
    with es:
        pc = es.enter_context(tc.tile_pool(name="const", bufs=1))
        pat = es.enter_context(tc.tile_pool(name="at", bufs=1))
        pkqv = es.enter_context(tc.tile_pool(name="kqv", bufs=1))
        pwf = es.enter_context(tc.tile_pool(name="wpan8", bufs=4))
        pwb = es.enter_context(tc.tile_pool(name="wpanb", bufs=2))
        pP = es.enter_context(tc.tile_pool(name="pP", bufs=6))
        p3 = es.enter_context(tc.tile_pool(name="p3", bufs=2))
        ppj = es.enter_context(tc.tile_pool(name="ps_proj", bufs=2, space="PSUM"))

        # --- persistent tensors -------------------------------------------
        xT8 = pkqv.tile([128, ET, SEQ], FP8, tag="xT8")
        K8 = pkqv.tile([128, ET, SEQ], FP8, tag="K8")
        Q8 = pkqv.tile([128, ET, 2, NQ], FP8, tag="Q8")
        # VO[kt, hp, slot, 128]: slot 2h+r = [ones/32 | V_{hi,lo} of head h]
        VO = pkqv.tile([128, TT, HP, 4, 128], FP8, tag="VO")
        AT4 = [
            pat.tile([128, 2, NQ], BF16, tag=f"at{j}", name=f"at{j}")
            for j in range(4)
        ]
        bqT = pc.tile([128, ET], F32, tag="bqT")
        boB = pc.tile([128, E], F32, tag="boB")

        # xTb streams in per-token-tile slices (v_chain tt only reads its own)
        pxtb = es.enter_context(tc.tile_pool(name="xtb", bufs=4))

        # --- input DMAs (sync queue, HWDGE) -------------------------------
        xT8_r = xT8_d.rearrange("(c p) t -> p c t", p=128)
        xTb_r = xTb_d.rearrange("(c p) t -> p c t", p=128)

        def load_panel(pool, src_w, pc0, name, dt, tag):
            wp = pool.tile([128, ET, 512], dt, tag=tag, name=name)
            nc.sync.dma_start(
                out=wp,
                in_=src_w.rearrange("(c p) n -> p c n", p=128)[
                    :, :, pc0 : pc0 + 512
                ],
            )
            return wp

        def load_xtb(panel, tt):
            xtb = pxtb.tile(
                [128, ET, 128], BF16, tag="xtb", name=f"xtb{panel}_{tt}"
            )
            nc.sync.dma_start(
                out=xtb, in_=xTb_r[:, :, tt * 128 : (tt + 1) * 128]
            )
            return xtb

        wp_v0 = load_panel(pwb, wvb_d, 0, "wpv0", BF16, "wpb")
        up_xtb = [load_xtb(0, tt) for tt in range(3)]
        for tb in range(4):
            nc.sync.dma_start(
                out=xT8[:, :, tb * 512 : (tb + 1) * 512],
                in_=xT8_r[:, :, tb * 512 : (tb + 1) * 512],
            )
            if tb == 0:
                wp_k0h = load_panel(pwf, wqkh_d, E, "wpk0h", FP8, "wpf")
                wp_k0l = load_panel(pwf, wqkl_d, E, "wpk0l", FP8, "wpf")
                up_xtb += [load_xtb(0, tt) for tt in range(3, 6)]
            if tb == 1:
                wp_q0h = load_panel(pwf, wqkh_d, 0, "wpq0h", FP8, "wpf")
                wp_q0l = load_panel(pwf, wqkl_d, 0, "wpq0l", FP8, "wpf")
        nc.sync.dma_start(out=bqT, in_=bqT_d)
        boB_bcast = bass.AP(
            tensor=boB_d.tensor, offset=boB_d.offset, ap=[[0, 128]] + boB_d.ap
        )
        nc.sync.dma_start(out=boB, in_=boB_bcast)

        # --- VO ones blocks (Pool engine) ---------------------------------
        for kt in range(TT):
            nc.gpsimd.memset(VO[:, kt, :, :, 0:64], 0.03125)

        # --- chain builders ------------------------------------------------
        def kq_half(ps, wp, ct, tb, start, stop):
            for eh in range(4):
                nc.tensor.matmul(
                    ps,
                    lhsT=wp[:, 2 * eh : 2 * eh + 2, ct * 128 : (ct + 1) * 128],
                    rhs=xT8[:, 2 * eh : 2 * eh + 2, tb * 512 : (tb + 1) * 512],
                    start=(start and eh == 0),
                    stop=(stop and eh == 3),
                    perf_mode=PM.DoubleRow,
                )

        def kq_chain_parts(wph, wpl, kind, panel, ct, tb):
            """Two sub-emissions so PE bursts stay short."""
            gct = panel * 4 + ct
            box = [None]

            def a():
                box[0] = ppj.tile(
                    [128, 512], F32, tag="ps", name=f"pskq{panel}_{kind}{ct}_{tb}"
                )
                kq_half(box[0], wph, ct, tb, True, False)

            def bfn():
                kq_half(box[0], wpl, ct, tb, False, True)
                kq_out(box[0], kind, gct, tb)

            return [a, bfn]

        def kq_out(ps, kind, gct, tb):
            if kind == "k":
                nc.vector.tensor_copy(
                    K8[:, gct, tb * 512 : (tb + 1) * 512], ps
                )
            else:
                nc.vector.tensor_scalar_add(
                    Q8[:, gct, 0, tb * 512 : (tb + 1) * 512],
                    ps,
                    bqT[:, gct : gct + 1],
                )
                nc.vector.scalar_tensor_tensor(
                    out=Q8[:, gct, 1, tb * 512 : (tb + 1) * 512],
                    in0=ps,
                    scalar=bqT[:, gct : gct + 1],
                    in1=Q8[:, gct, 0, tb * 512 : (tb + 1) * 512],
                    op0=mybir.AluOpType.add,
                    op1=mybir.AluOpType.subtract,
                )

        def v_chain_parts(wvp, panel, tt, xtb=None):
            """Three sub-emissions so PE bursts stay short."""
            box = [None, xtb]

            def a():
                if box[1] is None:
                    box[1] = load_xtb(panel, tt)
                box[0] = ppj.tile(
                    [128, 512], F32, tag="ps", name=f"psv{panel}_{tt}"
                )
                for ee in range(3):
                    nc.tensor.matmul(
                        box[0],
                        lhsT=box[1][:, ee, :],
                        rhs=wvp[:, ee, :],
                        start=(ee == 0),
                        stop=False,
                    )

            def bfn():
                for ee in range(3, 6):
                    nc.tensor.matmul(
                        box[0],
                        lhsT=box[1][:, ee, :],
                        rhs=wvp[:, ee, :],
                        start=False,
                        stop=False,
                    )

            def cfn():
                for ee in range(6, ET):
                    nc.tensor.matmul(
                        box[0],
                        lhsT=box[1][:, ee, :],
                        rhs=wvp[:, ee, :],
                        start=False,
                        stop=(ee == ET - 1),
                    )
                ps3 = box[0].rearrange("p (hl hd d) -> p hl hd d", hd=2, d=64)
                p0 = panel * 4
                vo_b = VO[:, tt, p0, 0, :]
                hi = bass.AP(
                    tensor=vo_b.tensor,
                    offset=vo_b.offset + 64,
                    ap=[vo_b.ap[0], [512, 4], [256, 2], [1, 64]],
                )
                lo = bass.AP(
                    tensor=vo_b.tensor,
                    offset=vo_b.offset + 128 + 64,
                    ap=[vo_b.ap[0], [512, 4], [256, 2], [1, 64]],
                )
                nc.vector.tensor_copy(hi, ps3)
                nc.vector.tensor_sub(lo, ps3, hi)

            return [a, bfn, cfn]

        # --- output projection (bf16) --------------------------------------
        wop = [None, None]

        def outproj_parts(tt, half):
            box = [None]

            def mk(j0, j1):
                def f():
                    if j0 == 0:
                        box[0] = ppj.tile(
                            [128, 512], F32, tag="ps", name=f"psop{tt}_{half}"
                        )
                    for j in range(j0, j1):
                        for i in range(2):
                            nc.tensor.matmul(
                                box[0],
                                lhsT=AT4[j][:, i, tt * 128 : (tt + 1) * 128],
                                rhs=wop[half][:, 2 * j + i, :],
                                start=(j == 0 and i == 0),
                                stop=(j == 3 and i == 1),
                            )
                    if j1 == 4:
                        outproj_fin(box[0], half, tt)
                return f

            return [mk(0, 2), mk(2, 4)]

        def outproj_chain(tt, half):
            for f in outproj_parts(tt, half):
                f()

        def outproj_fin(ps, half, tt):
            c0 = half * 512
            osb = p3.tile([128, 512], F32, tag="osb", bufs=2)
            nc.vector.scalar_tensor_tensor(
                out=osb,
                in0=ps,
                scalar=1.0 / 512.0,
                in1=boB[:, c0 : c0 + 512],
                op0=mybir.AluOpType.mult,
                op1=mybir.AluOpType.add,
            )
            nc.sync.dma_start(
                out=out[tt * 128 : (tt + 1) * 128, c0 : c0 + 512], in_=osb
            )

        # --- upfront chains -------------------------------------------------
        def run_all(parts):
            for f in parts:
                f()

        def kq_chain(*a, **k):
            return lambda: run_all(kq_chain_parts(*a, **k))

        def v_chain(*a, **k):
            return lambda: run_all(v_chain_parts(*a, **k))

        for tt in range(6):
            run_all(v_chain_parts(wp_v0, 0, tt, xtb=up_xtb[tt]))
        for tb in range(4):
            run_all(kq_chain_parts(wp_k0h, wp_k0l, "k", 0, 0, tb))
        run_all(kq_chain_parts(wp_q0h, wp_q0l, "q", 0, 0, 0))

        # --- deferred chain queue (deadline, fn), emitted 1 per kt slot ----
        panels = {}
        deferred = []

        def defer(dl, fn):
            deferred.append((dl, fn))

        for ct in (1, 2, 3):
            for tb in range(4):
                defer(2 * ct, lambda c=ct, t=tb: kq_chain(wp_k0h, wp_k0l, "k", 0, c, t)())
        for g in range(1, 4):
            for qb in range(2):
                defer(2 * g + qb, lambda c=g, q=qb: kq_chain(wp_q0h, wp_q0l, "q", 0, c, q)())
        defer(1, lambda: kq_chain(wp_q0h, wp_q0l, "q", 0, 0, 1)())
        # v1 chains: 8 early (deadlines 5-7), 8 inner in block 8
        for i in range(8):
            defer(5 + i // 3, lambda t=i: v_chain(panels["v1"], 1, t)())
        for ct in range(4):
            for tb in range(4):
                defer(8 + 2 * ct, lambda c=ct, t=tb: kq_chain(panels["k1h"], panels["k1l"], "k", 1, c, t)())
        for g in range(4):
            for qb in range(2):
                defer(8 + 2 * g + qb, lambda c=g, q=qb: kq_chain(panels["q1h"], panels["q1l"], "q", 1, c, q)())
        deferred.sort(key=lambda t: t[0])

        # --- attention ------------------------------------------------------
        att_es = ExitStack()
        pss = att_es.enter_context(tc.tile_pool(name="ps_s", bufs=2, space="PSUM"))
        pav = att_es.enter_context(tc.tile_pool(name="ps_av", bufs=1, space="PSUM"))

        def normalize(hp, qb, av):
            q0 = qb * 512
            rec = p3.tile([64, 1024], F32, tag="rec", bufs=2)
            nc.vector.reciprocal_approx_fast(rec, av[0:64, :])
            j, i = hp // 2, hp % 2
            nc.vector.tensor_mul(
                AT4[j][0:64, i, q0 : q0 + 512], av[64:128, 0:512], rec[:, 0:512]
            )
            nc.vector.tensor_mul(
                AT4[j][64:128, i, q0 : q0 + 512],
                av[64:128, 512:1024],
                rec[:, 512:1024],
            )

        def attention_block(b, hp, qb, inner):
            q0 = qb * 512
            av_box = [None]
            Pt = [None]
            dve_kts = _dve_kts(b)
            pending_av = []

            def emit_av(kt, P):
                if av_box[0] is None:
                    av_box[0] = pav.tile([128, 1024], F32, tag="av", name=f"av{b}")
                av = av_box[0]
                for h in range(2):
                    pslot = P[:, h, kt % 2, :]
                    rhs = bass.AP(
                        tensor=pslot.tensor,
                        offset=pslot.offset,
                        ap=[pslot.ap[0], [0, 2], pslot.ap[-1]],
                    )
                    nc.tensor.matmul(
                        av[:, h * 512 : (h + 1) * 512],
                        lhsT=VO[:, kt, hp, 2 * h : 2 * h + 2, :],
                        rhs=rhs,
                        start=(kt == 0),
                        stop=(kt == TT - 1),
                        perf_mode=PM.DoubleRow,
                    )

            for kt in range(TT):
                for fn in inner.get(kt, ()):
                    fn()
                ktm = kt % 2
                if ktm == 0:
                    Pt[0] = pP.tile(
                        [128, 2, 2, 512], FP8, tag="P", name=f"P{b}_{kt}"
                    )
                P = Pt[0]
                ps = pss.tile([128, 1024], F32, tag="ps_s")
                for h in range(2):
                    r0 = 64 * h
                    kbase = K8[r0 : r0 + 64, hp, kt * 128 : (kt + 1) * 128]
                    lhsT = bass.AP(
                        tensor=kbase.tensor,
                        offset=kbase.offset,
                        ap=[kbase.ap[0], [0, 2], kbase.ap[-1]],
                    )
                    nc.tensor.matmul(
                        ps[:, h * 512 : (h + 1) * 512],
                        lhsT=lhsT,
                        rhs=Q8[r0 : r0 + 64, hp, :, q0 : q0 + 512],
                        start=True,
                        stop=True,
                        perf_mode=PM.DoubleRow,
                    )
                if kt in dve_kts:
                    nc.vector.tensor_scalar(
                        out=P[:, :, ktm, :].bitcast(I8),
                        in0=ps,
                        scalar1=C1,
                        scalar2=C2,
                        op0=mybir.AluOpType.mult,
                        op1=mybir.AluOpType.add,
                    )
                else:
                    nc.scalar.activation(P[:, :, ktm, :], ps, AF.Exp, scale=SC)
                pending_av.append((kt, P))
                if len(pending_av) > 5:
                    emit_av(*pending_av.pop(0))

            def finish():
                while pending_av:
                    emit_av(*pending_av.pop(0))
                return av_box[0]

            return finish

        blocks = [(hp, qb) for hp in range(HP) for qb in range(QB)]
        prev_fin = None
        di = 0
        for b, (hp, qb) in enumerate(blocks):
            carry = []
            if prev_fin is not None:
                phff, php, pqb = prev_fin
                carry = [
                    lambda f=phff: f(),
                    lambda f=phff, h=php, q=pqb: normalize(h, q, f()),
                ]
                prev_fin = None
            if b == 3:
                panels["v1"] = load_panel(pwb, wvb_d, 512, "wpv1", BF16, "wpb")
            if b == 5:
                panels["k1h"] = load_panel(pwf, wqkh_d, E + 512, "wpk1h", FP8, "wpf")
                panels["k1l"] = load_panel(pwf, wqkl_d, E + 512, "wpk1l", FP8, "wpf")
            if b == 6:
                panels["q1h"] = load_panel(pwf, wqkh_d, 512, "wpq1h", FP8, "wpf")
                panels["q1l"] = load_panel(pwf, wqkl_d, 512, "wpq1l", FP8, "wpf")
            if b == 11:
                # reuse the bf16 panel pool (rotation deps make this safe)
                wop[0] = load_panel(pwb, wob_d, 0, "wpo0", BF16, "wpb")
                wop[1] = load_panel(pwb, wob_d, 512, "wpo1", BF16, "wpb")

            # schedule this block's inner work (whole chains per slot)
            inner = {}
            slot_fns = []
            if b == 0:
                for tt in range(6, TT):
                    slot_fns.append(lambda t=tt: run_all(v_chain_parts(wp_v0, 0, t)))
            if b == 8:
                for tt in range(8, TT):
                    slot_fns.append(
                        lambda t=tt: run_all(v_chain_parts(panels["v1"], 1, t))
                    )
            while di < len(deferred) and deferred[di][0] <= b + 1:
                slot_fns.append(deferred[di][1])
                di += 1
            if b == 15:
                for tt in range(4):
                    for half in range(2):
                        slot_fns.append(lambda t=tt, hf=half: outproj_chain(t, hf))
            if b in (0, 8):
                order = list(range(TT))  # v blocks: fill every slot in order
            else:
                order = [3, 7, 11, 13, 14, 15, 2, 4, 5, 6, 8, 9, 10, 12]
            for i, fn in enumerate(slot_fns):
                inner.setdefault(order[i % len(order)], []).append(fn)
            if carry:
                inner.setdefault(1, []).insert(0, carry[0])
                inner.setdefault(2, []).insert(0, carry[1])

            fin = attention_block(b, hp, qb, inner)
            prev_fin = (fin, hp, qb)
            if b >= 14:
                # flush+normalize immediately: block 15's inner outproj (qb0)
                # and the tail (qb1) read these AT4 columns
                normalize(hp, qb, fin())
                prev_fin = None
        assert di == len(deferred), (di, len(deferred))
        for tt in range(4, 8):
            for half in range(2):
                outproj_chain(tt, half)
        att_es.close()


_NC = None


def _get_program():
    global _NC
    if _NC is None:
        _NC = build_program()
    return _NC


def make_in_maps(x, Wqkv, bqkv, Wo, bo):
    np8 = mybir.dt.np(FP8)
    npb = mybir.dt.np(BF16)
    Wqkv = np.asarray(Wqkv, np.float32)
    Wo = np.asarray(Wo, np.float32)
    bqkv = np.asarray(bqkv, np.float32)
    bo = np.asarray(bo, np.float32)
    wqk32 = 32.0 * Wqkv[:, 0 : 2 * E]
    wqk_hi = wqk32.astype(np8)
    wqk_lo = (wqk32 - wqk_hi.astype(np.float32)).astype(np8)
    w = {
        "wqk_hi": np.ascontiguousarray(wqk_hi),
        "wqk_lo": np.ascontiguousarray(wqk_lo),
        "wv_b": np.ascontiguousarray((32.0 * Wqkv[:, 2 * E :]).astype(npb)),
        "wo_b": np.ascontiguousarray(Wo.astype(npb)),
        "bqT": np.ascontiguousarray(
            (32.0 * bqkv[0:E]).reshape(ET, 128).T.astype(np.float32)
        ),
        "boB": np.ascontiguousarray((bqkv[2 * E :] @ Wo + bo).astype(np.float32)),
    }
    x = np.asarray(x, np.float32)
    in_maps = []
    for c in range(N_CORES):
        b, s = divmod(c, 2)
        xb = x[b]
        if s == 1:
            xb = np.roll(xb, -NQ, axis=0)
        xt = np.ascontiguousarray(xb.T)
        in_maps.append(
            {
                "xT8": np.ascontiguousarray(xt.astype(np8)),
                "xTb": np.ascontiguousarray(xt.astype(npb)),
                **w,
            }
        )
    return in_maps


def gather_out(results):
    out = np.empty((4, SEQ, E), np.float32)
    for c in range(N_CORES):
        b, s = divmod(c, 2)
        out[b, s * NQ : (s + 1) * NQ] = results[c]["out"]
    return out


def kernel(x, Wqkv, bqkv, Wo, bo):
    from concourse.bass_utils import run_bass_kernel_spmd

    nc = _get_program()
    in_maps = make_in_maps(x, Wqkv, bqkv, Wo, bo)
    res = run_bass_kernel_spmd(nc, in_maps, core_ids=list(range(N_CORES)))
    return gather_out(res.results)


# revision 29
# speedup vs baseline: 1.0014x; 1.0014x over previous
"""Trainium2 Bass kernel for a 16-head MHA layer (batch 4, seq 2048, embed 1024).

Sharding: 8 cores; core c handles batch c//2 and query-token half c%2, with the
core's x rotated so its 1024 query tokens sit in rows 0:1024 (softmax is
permutation-invariant over key order). K/V cover the full sequence on-core; no
collectives. Weights replicated.

Numerics: every hot matmul runs in fp8e4 DoubleRow (2 contraction slots per
matmul at 0.5 cycles/output-column), with the residual-pair trick soaking up
fp8 quantization error wherever a slot is free:
 - QK projections contract (W_hi, W_lo) fp8 pairs of 32*Wqkv (scaled out of
   the fp8-subnormal range; host-prepared).
 - Scores contract d=64 with slots (Q_hi, Q_lo) -- the Q store residual --
   against a stride-0-duplicated K.
 - AV contracts slots ([ones/32|V_hi], [ones/32|V_lo]) against stride-0-
   duplicated P, so each matmul emits denominator/16 in rows 0:64 and the
   V-residual-corrected AV in rows 64:128.
 - V projection and the output projection run in bf16 (x^T is uploaded twice:
   fp8 for QK chains, bf16 for V chains; out-proj reads bf16 AT tiles).
 - K bias is dropped (softmax-invariant); V bias + output bias fold into a
   host-precomputed boB = b_v @ Wo + bo; Q bias (x32) is added on-chip.
 - exp: mostly exact on the ACT engine (fp8 out, scale folded); a tunable set
   of key-tiles runs a one-instruction DVE Schraudolph (int8 bit pattern) exp
   to split the exp load across both engines.
"""

import sys

for _p in ("/opt/trn_rl_repo",):
    if _p not in sys.path:
        sys.path.insert(0, _p)

import numpy as np

import concourse.bass as bass  # noqa: E402
import concourse.mybir as mybir  # noqa: E402
import concourse.tile as tile  # noqa: E402
from concourse import bacc  # noqa: E402

SEQ = 2048
E = 1024
H = 16
D = 64
NQ = 1024  # query tokens per core
N_CORES = 8

ET = E // 128  # 8 e-chunks
TT = SEQ // 128  # 16 key/token tiles
HP = H // 2  # 8 head pairs
QB = NQ // 512  # 2 query blocks

F32 = mybir.dt.float32
BF16 = mybir.dt.bfloat16
FP8 = mybir.dt.float8e4
I8 = mybir.dt.int8
AF = mybir.ActivationFunctionType
PM = mybir.MatmulPerfMode

# exp(s_fp8 * SC) == exp(s_true * 0.125); fp8 QK weights carry 32x each
SC = 0.125 / 1024.0
# Schraudolph: int8 bits = round(s_fp8*C1 + C2) reinterpreted as fp8e4
C1 = 8 * 1.4426950408889634 * SC
C2 = 56.0 - 8 * 0.043


def _dve_kts(b):
    if b in (0, 8):  # these blocks' DVE is busy with inner V copies
        return ()
    return (2, 5, 8, 11, 14)


def build_program():
    nc = bacc.Bacc(trn_type="TRN2", target_bir_lowering=False, debug=False)

    xT8 = nc.dram_tensor("xT8", [E, SEQ], FP8, kind="ExternalInput").ap()
    xTb = nc.dram_tensor("xTb", [E, SEQ], BF16, kind="ExternalInput").ap()
    wqk_hi = nc.dram_tensor("wqk_hi", [E, 2 * E], FP8, kind="ExternalInput").ap()
    wqk_lo = nc.dram_tensor("wqk_lo", [E, 2 * E], FP8, kind="ExternalInput").ap()
    wv_b = nc.dram_tensor("wv_b", [E, E], BF16, kind="ExternalInput").ap()
    wo_b = nc.dram_tensor("wo_b", [E, E], BF16, kind="ExternalInput").ap()
    bqT = nc.dram_tensor("bqT", [128, ET], F32, kind="ExternalInput").ap()
    boB = nc.dram_tensor("boB", [E], F32, kind="ExternalInput").ap()
    out = nc.dram_tensor("out", [NQ, E], F32, kind="ExternalOutput").ap()

    with tile.TileContext(nc) as tc:
        _body(nc, tc, xT8, xTb, wqk_hi, wqk_lo, wv_b, wo_b, bqT, boB, out)

    nc.compile()
    return nc


def _body(nc, tc, xT8_d, xTb_d, wqkh_d, wqkl_d, wvb_d, wob_d, bqT_d, boB_d, out):
    from contextlib import ExitStack

    es = ExitStack()
    with es:
        pc = es.enter_context(tc.tile_pool(name="const", bufs=1))
        pat = es.enter_context(tc.tile_pool(name="at", bufs=1))
        pkqv = es.enter_context(tc.tile_pool(name="kqv", bufs=1))
        pwf = es.enter_context(tc.tile_pool(name="wpan8", bufs=4))
        pwb = es.enter_context(tc.tile_pool(name="wpanb", bufs=2))
        pP = es.enter_context(tc.tile_pool(name="pP", bufs=6))
        p3 = es.enter_context(tc.tile_pool(name="p3", bufs=2))
        ppj = es.enter_context(tc.tile_pool(name="ps_proj", bufs=2, space="PSUM"))

        # --- persistent tensors -------------------------------------------
        xT8 = pkqv.tile([128, ET, SEQ], FP8, tag="xT8")
        K8 = pkqv.tile([128, ET, SEQ], FP8, tag="K8")
        Q8 = pkqv.tile([128, ET, 2, NQ], FP8, tag="Q8")
        # VO[kt, hp, slot, 128]: slot 2h+r = [ones/32 | V_{hi,lo} of head h]
        VO = pkqv.tile([128, TT, HP, 4, 128], FP8, tag="VO")
        AT4 = [
            pat.tile([128, 2, NQ], BF16, tag=f"at{j}", name=f"at{j}")
            for j in range(4)
        ]
        bqT = pc.tile([128, ET], F32, tag="bqT")
        boB = pc.tile([128, E], F32, tag="boB")

        # xTb streams in per-token-tile slices (v_chain tt only reads its own)
        pxtb = es.enter_context(tc.tile_pool(name="xtb", bufs=4))

        # --- input DMAs (sync queue, HWDGE) -------------------------------
        xT8_r = xT8_d.rearrange("(c p) t -> p c t", p=128)
        xTb_r = xTb_d.rearrange("(c p) t -> p c t", p=128)

        def load_panel(pool, src_w, pc0, name, dt, tag):
            wp = pool.tile([128, ET, 512], dt, tag=tag, name=name)
            nc.sync.dma_start(
                out=wp,
                in_=src_w.rearrange("(c p) n -> p c n", p=128)[
                    :, :, pc0 : pc0 + 512
                ],
            )
            return wp

        def load_xtb(panel, tt):
            xtb = pxtb.tile(
                [128, ET, 128], BF16, tag="xtb", name=f"xtb{panel}_{tt}"
            )
            nc.sync.dma_start(
                out=xtb, in_=xTb_r[:, :, tt * 128 : (tt + 1) * 128]
            )
            return xtb

        wp_v0 = load_panel(pwb, wvb_d, 0, "wpv0", BF16, "wpb")
        up_xtb = [load_xtb(0, tt) for tt in range(3)]
        for tb in range(4):
            nc.sync.dma_start(
                out=xT8[:, :, tb * 512 : (tb + 1) * 512],
                in_=xT8_r[:, :, tb * 512 : (tb + 1) * 512],
            )
            if tb == 0:
                wp_k0h = load_panel(pwf, wqkh_d, E, "wpk0h", FP8, "wpf")
                wp_k0l = load_panel(pwf, wqkl_d, E, "wpk0l", FP8, "wpf")
                up_xtb += [load_xtb(0, tt) for tt in range(3, 6)]
            if tb == 1:
                wp_q0h = load_panel(pwf, wqkh_d, 0, "wpq0h", FP8, "wpf")
                wp_q0l = load_panel(pwf, wqkl_d, 0, "wpq0l", FP8, "wpf")
        nc.sync.dma_start(out=bqT, in_=bqT_d)
        boB_bcast = bass.AP(
            tensor=boB_d.tensor, offset=boB_d.offset, ap=[[0, 128]] + boB_d.ap
        )
        nc.sync.dma_start(out=boB, in_=boB_bcast)

        # --- VO ones blocks (Pool engine) ---------------------------------
        for kt in range(TT):
            nc.gpsimd.memset(VO[:, kt, :, :, 0:64], 0.03125)

        # --- chain builders ------------------------------------------------
        def kq_half(ps, wp, ct, tb, start, stop):
            for eh in range(4):
                nc.tensor.matmul(
                    ps,
                    lhsT=wp[:, 2 * eh : 2 * eh + 2, ct * 128 : (ct + 1) * 128],
                    rhs=xT8[:, 2 * eh : 2 * eh + 2, tb * 512 : (tb + 1) * 512],
                    start=(start and eh == 0),
                    stop=(stop and eh == 3),
                    perf_mode=PM.DoubleRow,
                )

        def kq_chain_parts(wph, wpl, kind, panel, ct, tb):
            """Two sub-emissions so PE bursts stay short."""
            gct = panel * 4 + ct
            box = [None]

            def a():
                box[0] = ppj.tile(
                    [128, 512], F32, tag="ps", name=f"pskq{panel}_{kind}{ct}_{tb}"
                )
                kq_half(box[0], wph, ct, tb, True, False)

            def bfn():
                kq_half(box[0], wpl, ct, tb, False, True)
                kq_out(box[0], kind, gct, tb)

            return [a, bfn]

        def kq_out(ps, kind, gct, tb):
            if kind == "k":
                nc.vector.tensor_copy(
                    K8[:, gct, tb * 512 : (tb + 1) * 512], ps
                )
            else:
                nc.vector.tensor_scalar_add(
                    Q8[:, gct, 0, tb * 512 : (tb + 1) * 512],
                    ps,
                    bqT[:, gct : gct + 1],
                )
                nc.vector.scalar_tensor_tensor(
                    out=Q8[:, gct, 1, tb * 512 : (tb + 1) * 512],
                    in0=ps,
                    scalar=bqT[:, gct : gct + 1],
                    in1=Q8[:, gct, 0, tb * 512 : (tb + 1) * 512],
                    op0=mybir.AluOpType.add,
                    op1=mybir.AluOpType.subtract,
                )

        def v_chain_parts(wvp, panel, tt, xtb=None):
            """Three sub-emissions so PE bursts stay short."""
            box = [None, xtb]

            def a():
                if box[1] is None:
                    box[1] = load_xtb(panel, tt)
                box[0] = ppj.tile(
                    [128, 512], F32, tag="ps", name=f"psv{panel}_{tt}"
                )
                for ee in range(3):
                    nc.tensor.matmul(
                        box[0],
                        lhsT=box[1][:, ee, :],
                        rhs=wvp[:, ee, :],
                        start=(ee == 0),
                        stop=False,
                    )

            def bfn():
                for ee in range(3, 6):
                    nc.tensor.matmul(
                        box[0],
                        lhsT=box[1][:, ee, :],
                        rhs=wvp[:, ee, :],
                        start=False,
                        stop=False,
                    )

            def cfn():
                for ee in range(6, ET):
                    nc.tensor.matmul(
                        box[0],
                        lhsT=box[1][:, ee, :],
                        rhs=wvp[:, ee, :],
                        start=False,
                        stop=(ee == ET - 1),
                    )
                ps3 = box[0].rearrange("p (hl hd d) -> p hl hd d", hd=2, d=64)
                p0 = panel * 4
                vo_b = VO[:, tt, p0, 0, :]
                hi = bass.AP(
                    tensor=vo_b.tensor,
                    offset=vo_b.offset + 64,
                    ap=[vo_b.ap[0], [512, 4], [256, 2], [1, 64]],
                )
                lo = bass.AP(
                    tensor=vo_b.tensor,
                    offset=vo_b.offset + 128 + 64,
                    ap=[vo_b.ap[0], [512, 4], [256, 2], [1, 64]],
                )
                nc.vector.tensor_copy(hi, ps3)
                nc.vector.tensor_sub(lo, ps3, hi)

            return [a, bfn, cfn]

        # --- output projection (bf16) --------------------------------------
        wop = [None, None]

        def outproj_parts(tt, half):
            box = [None]

            def mk(j0, j1):
                def f():
                    if j0 == 0:
                        box[0] = ppj.tile(
                            [128, 512], F32, tag="ps", name=f"psop{tt}_{half}"
                        )
                    for j in range(j0, j1):
                        for i in range(2):
                            nc.tensor.matmul(
                                box[0],
                                lhsT=AT4[j][:, i, tt * 128 : (tt + 1) * 128],
                                rhs=wop[half][:, 2 * j + i, :],
                                start=(j == 0 and i == 0),
                                stop=(j == 3 and i == 1),
                            )
                    if j1 == 4:
                        outproj_fin(box[0], half, tt)
                return f

            return [mk(0, 2), mk(2, 4)]

        def outproj_chain(tt, half):
            for f in outproj_parts(tt, half):
                f()

        def outproj_fin(ps, half, tt):
            c0 = half * 512
            osb = p3.tile([128, 512], F32, tag="osb", bufs=2)
            nc.vector.scalar_tensor_tensor(
                out=osb,
                in0=ps,
                scalar=1.0 / 512.0,
                in1=boB[:, c0 : c0 + 512],
                op0=mybir.AluOpType.mult,
                op1=mybir.AluOpType.add,
            )
            nc.sync.dma_start(
                out=out[tt * 128 : (tt + 1) * 128, c0 : c0 + 512], in_=osb
            )

        # --- upfront chains -------------------------------------------------
        def run_all(parts):
            for f in parts:
                f()

        def kq_chain(*a, **k):
            return lambda: run_all(kq_chain_parts(*a, **k))

        def v_chain(*a, **k):
            return lambda: run_all(v_chain_parts(*a, **k))

        for tt in range(6):
            run_all(v_chain_parts(wp_v0, 0, tt, xtb=up_xtb[tt]))
        for tb in range(4):
            run_all(kq_chain_parts(wp_k0h, wp_k0l, "k", 0, 0, tb))
        run_all(kq_chain_parts(wp_q0h, wp_q0l, "q", 0, 0, 0))

        # --- deferred chain queue (deadline, fn), emitted 1 per kt slot ----
        panels = {}
        deferred = []

        def defer(dl, fn):
            deferred.append((dl, fn))

        for ct in (1, 2, 3):
            for tb in range(4):
                defer(2 * ct, lambda c=ct, t=tb: kq_chain(wp_k0h, wp_k0l, "k", 0, c, t)())
        for g in range(1, 4):
            for qb in range(2):
                defer(2 * g + qb, lambda c=g, q=qb: kq_chain(wp_q0h, wp_q0l, "q", 0, c, q)())
        defer(1, lambda: kq_chain(wp_q0h, wp_q0l, "q", 0, 0, 1)())
        # v1 chains: 8 early (deadlines 5-7), 8 inner in block 8
        for i in range(8):
            defer(5 + i // 3, lambda t=i: v_chain(panels["v1"], 1, t)())
        for ct in range(4):
            for tb in range(4):
                defer(8 + 2 * ct, lambda c=ct, t=tb: kq_chain(panels["k1h"], panels["k1l"], "k", 1, c, t)())
        for g in range(4):
            for qb in range(2):
                defer(8 + 2 * g + qb, lambda c=g, q=qb: kq_chain(panels["q1h"], panels["q1l"], "q", 1, c, q)())
        deferred.sort(key=lambda t: t[0])

        # --- attention ------------------------------------------------------
        att_es = ExitStack()
        pss = att_es.enter_context(tc.tile_pool(name="ps_s", bufs=2, space="PSUM"))
        pav = att_es.enter_context(tc.tile_pool(name="ps_av", bufs=1, space="PSUM"))

        def normalize(hp, qb, av):
            q0 = qb * 512
            rec = p3.tile([64, 1024], F32, tag="rec", bufs=2)
            nc.vector.reciprocal_approx_fast(rec, av[0:64, :])
            j, i = hp // 2, hp % 2
            nc.vector.tensor_mul(
                AT4[j][0:64, i, q0 : q0 + 512], av[64:128, 0:512], rec[:, 0:512]
            )
            nc.vector.tensor_mul(
                AT4[j][64:128, i, q0 : q0 + 512],
                av[64:128, 512:1024],
                rec[:, 512:1024],
            )

        def attention_block(b, hp, qb, inner):
            q0 = qb * 512
            av = pav.tile([128, 1024], F32, tag="av", name=f"av{b}")
            Pt = [None]
            dve_kts = _dve_kts(b)
            pending_av = []

            def emit_av(kt, P):
                for h in range(2):
                    pslot = P[:, h, kt % 2, :]
                    rhs = bass.AP(
                        tensor=pslot.tensor,
                        offset=pslot.offset,
                        ap=[pslot.ap[0], [0, 2], pslot.ap[-1]],
                    )
                    nc.tensor.matmul(
                        av[:, h * 512 : (h + 1) * 512],
                        lhsT=VO[:, kt, hp, 2 * h : 2 * h + 2, :],
                        rhs=rhs,
                        start=(kt == 0),
                        stop=(kt == TT - 1),
                        perf_mode=PM.DoubleRow,
                    )

            for kt in range(TT):
                for fn in inner.get(kt, ()):
                    fn()
                ktm = kt % 2
                if ktm == 0:
                    Pt[0] = pP.tile(
                        [128, 2, 2, 512], FP8, tag="P", name=f"P{b}_{kt}"
                    )
                P = Pt[0]
                ps = pss.tile([128, 1024], F32, tag="ps_s")
                for h in range(2):
                    r0 = 64 * h
                    kbase = K8[r0 : r0 + 64, hp, kt * 128 : (kt + 1) * 128]
                    lhsT = bass.AP(
                        tensor=kbase.tensor,
                        offset=kbase.offset,
                        ap=[kbase.ap[0], [0, 2], kbase.ap[-1]],
                    )
                    nc.tensor.matmul(
                        ps[:, h * 512 : (h + 1) * 512],
                        lhsT=lhsT,
                        rhs=Q8[r0 : r0 + 64, hp, :, q0 : q0 + 512],
                        start=True,
                        stop=True,
                        perf_mode=PM.DoubleRow,
                    )
                if kt in dve_kts:
                    nc.vector.tensor_scalar(
                        out=P[:, :, ktm, :].bitcast(I8),
                        in0=ps,
                        scalar1=C1,
                        scalar2=C2,
                        op0=mybir.AluOpType.mult,
                        op1=mybir.AluOpType.add,
                    )
                else:
                    nc.scalar.activation(P[:, :, ktm, :], ps, AF.Exp, scale=SC)
                pending_av.append((kt, P))
                if len(pending_av) > 5:
                    emit_av(*pending_av.pop(0))
            for item in pending_av:
                emit_av(*item)
            return av

        blocks = [(hp, qb) for hp in range(HP) for qb in range(QB)]
        av_prev = None
        di = 0
        for b, (hp, qb) in enumerate(blocks):
            if av_prev is not None:
                normalize(*av_prev)
                av_prev = None
            if b == 3:
                panels["v1"] = load_panel(pwb, wvb_d, 512, "wpv1", BF16, "wpb")
            if b == 5:
                panels["k1h"] = load_panel(pwf, wqkh_d, E + 512, "wpk1h", FP8, "wpf")
                panels["k1l"] = load_panel(pwf, wqkl_d, E + 512, "wpk1l", FP8, "wpf")
            if b == 6:
                panels["q1h"] = load_panel(pwf, wqkh_d, 512, "wpq1h", FP8, "wpf")
                panels["q1l"] = load_panel(pwf, wqkl_d, 512, "wpq1l", FP8, "wpf")
            if b == 11:
                # reuse the bf16 panel pool (rotation deps make this safe)
                wop[0] = load_panel(pwb, wob_d, 0, "wpo0", BF16, "wpb")
                wop[1] = load_panel(pwb, wob_d, 512, "wpo1", BF16, "wpb")

            # schedule this block's inner work (whole chains per slot)
            inner = {}
            slot_fns = []
            if b == 0:
                for tt in range(6, TT):
                    slot_fns.append(lambda t=tt: run_all(v_chain_parts(wp_v0, 0, t)))
            if b == 8:
                for tt in range(8, TT):
                    slot_fns.append(
                        lambda t=tt: run_all(v_chain_parts(panels["v1"], 1, t))
                    )
            while di < len(deferred) and deferred[di][0] <= b + 1:
                slot_fns.append(deferred[di][1])
                di += 1
            if b == 15:
                for tt in range(4):
                    for half in range(2):
                        slot_fns.append(lambda t=tt, hf=half: outproj_chain(t, hf))
            if b in (0, 8):
                order = list(range(TT))  # v blocks: fill every slot in order
            else:
                order = [3, 7, 11, 13, 14, 15, 2, 4, 5, 6, 8, 9, 10, 12]
            for i, fn in enumerate(slot_fns):
                inner.setdefault(order[i % len(order)], []).append(fn)

            av = attention_block(b, hp, qb, inner)
            av_prev = (hp, qb, av)
            if b >= 14:
                # normalize immediately: block 15's inner outproj chains (qb0)
                # and the tail (qb1) read these AT4 columns
                normalize(*av_prev)
                av_prev = None
        assert di == len(deferred), (di, len(deferred))
        for tt in range(4, 8):
            for half in range(2):
                outproj_chain(tt, half)
        att_es.close()


_NC = None


def _get_program():
    global _NC
    if _NC is None:
        _NC = build_program()
    return _NC


def make_in_maps(x, Wqkv, bqkv, Wo, bo):
    np8 = mybir.dt.np(FP8)
    npb = mybir.dt.np(BF16)
    Wqkv = np.asarray(Wqkv, np.float32)
    Wo = np.asarray(Wo, np.float32)
    bqkv = np.asarray(bqkv, np.float32)
    bo = np.asarray(bo, np.float32)
    wqk32 = 32.0 * Wqkv[:, 0 : 2 * E]
    wqk_hi = wqk32.astype(np8)
    wqk_lo = (wqk32 - wqk_hi.astype(np.float32)).astype(np8)
    w = {
        "wqk_hi": np.ascontiguousarray(wqk_hi),
        "wqk_lo": np.ascontiguousarray(wqk_lo),
        "wv_b": np.ascontiguousarray((32.0 * Wqkv[:, 2 * E :]).astype(npb)),
        "wo_b": np.ascontiguousarray(Wo.astype(npb)),
        "bqT": np.ascontiguousarray(
            (32.0 * bqkv[0:E]).reshape(ET, 128).T.astype(np.float32)
        ),
        "boB": np.ascontiguousarray((bqkv[2 * E :] @ Wo + bo).astype(np.float32)),
    }
    x = np.asarray(x, np.float32)
    in_maps = []
    for c in range(N_CORES):
        b, s = divmod(c, 2)
        xb = x[b]
        if s == 1:
            xb = np.roll(xb, -NQ, axis=0)
        xt = np.ascontiguousarray(xb.T)
        in_maps.append(
            {
                "xT8": np.ascontiguousarray(xt.astype(np8)),
                "xTb": np.ascontiguousarray(xt.astype(npb)),
                **w,
            }
        )
    return in_maps


def gather_out(results):
    out = np.empty((4, SEQ, E), np.float32)
    for c in range(N_CORES):
        b, s = divmod(c, 2)
        out[b, s * NQ : (s + 1) * NQ] = results[c]["out"]
    return out


def kernel(x, Wqkv, bqkv, Wo, bo):
    from concourse.bass_utils import run_bass_kernel_spmd

    nc = _get_program()
    in_maps = make_in_maps(x, Wqkv, bqkv, Wo, bo)
    res = run_bass_kernel_spmd(nc, in_maps, core_ids=list(range(N_CORES)))
    return gather_out(res.results)
